# revision 10
# baseline (speedup 1.0000x reference)
"""Median graph convolution on 8 Trainium2 NeuronCores.

out[n, c] = median over valid neighbors j of (x @ kernel)[neighbors[n, j], c]
(lower median, rank (deg-1)//2 of the first deg neighbor slots).

Strategy (data-parallel over nodes, 6272 nodes/core):
  - host sorts nodes by degree (descending), striped across the 8 cores so
    every core sees the same degree profile and one compiled program fits all
  - each core matmuls its node shard on the PE -> h shard (fp16),
    AllGather into a per-core HBM table with trailing +inf sentinel rows
  - the table is indexed as 512-byte PAIR rows (two h rows per descriptor),
    so the 50176-row table needs only 25089 int16-indexable pair rows;
    each real neighbor costs exactly ONE gather descriptor
  - only the first maxdeg(tile) slots are gathered per 128-node tile
    (pads ride as +inf sentinel descriptors / vector memset)
  - a copy + copy_predicated (int16 parity mask, stride-0 broadcast over
    channels) selects the wanted half of each gathered pair
  - a degree-adaptive bitonic network sorts the two H-halves of the slot
    array and a rank-r two-way merge formula extracts the lower median
"""

import sys

sys.path.insert(0, "/opt/trn_rl_repo")

import numpy as np

N, K, IN_C, OUT_C = 50000, 32, 256, 128
NCORES = 8
NTILES = 49                      # 128-node tiles per core
SHARD = NTILES * 128             # 6272
NPAD = SHARD * NCORES            # 50176
TROWS = NPAD + 4                 # +inf sentinel rows at the end
SENT_PAIR = NPAD // 2            # pair index of the +inf sentinel row pair
NPAIRS = SENT_PAIR + 1           # pair rows addressable by the gather
GCHUNK = 16                      # slots per dma_gather call (16*128 = 2048 idx)
MAXSLOTS = 32

_CACHE = {}


def _next_pow2(x):
    p = 1
    while p < x:
        p *= 2
    return p


def _make_schedule(deg_sorted):
    """Per-tile (maxd, H, r_list) from the global descending degree profile."""
    sched = []
    for t in range(NTILES):
        degs = deg_sorted[t * 128 * NCORES:(t + 1) * 128 * NCORES]
        maxd = int(degs[0])
        H = max(1, _next_pow2(maxd) // 2)
        rs = sorted({int((d - 1) // 2) for d in degs}, reverse=True)
        sched.append((maxd, H, tuple(rs)))
    return tuple(sched)


def _emit_program(sched):
    import concourse.tile as tile
    import concourse.mybir as mybir
    from concourse import bacc
    from concourse.bass import AP
    from concourse.library_config import mlp

    fp16 = mybir.dt.float16
    fp32 = mybir.dt.float32
    i16 = mybir.dt.int16
    Alu = mybir.AluOpType

    tot_idx_cols = sum(maxd * 8 for (maxd, _, _) in sched)
    tot_par_cols = sum(maxd for (maxd, _, _) in sched)
    tot_pick = sum(len(rs) - 1 for (_, _, rs) in sched)

    nc = bacc.Bacc("TRN2", target_bir_lowering=False, num_swdge_queues=4,
                   dynamic_dma_scratch_size=32768)

    xT = nc.dram_tensor("xT", [IN_C, SHARD], fp16, kind="ExternalInput")
    w = nc.dram_tensor("w", [IN_C, OUT_C], fp16, kind="ExternalInput")
    idx_d = nc.dram_tensor("idx", [128, tot_idx_cols], i16, kind="ExternalInput")
    par_d = nc.dram_tensor("par", [128, tot_par_cols], i16, kind="ExternalInput")
    pick_d = nc.dram_tensor("pick", [128, max(1, tot_pick)], i16, kind="ExternalInput")
    infs = nc.dram_tensor("infs", [4, OUT_C], fp16, kind="ExternalInput")  # +inf rows
    out = nc.dram_tensor("out", [SHARD, OUT_C], fp32, kind="ExternalOutput")
    table = nc.dram_tensor("table", [TROWS, OUT_C], fp16)
    hshard = nc.dram_tensor("hshard", [SHARD, OUT_C], fp16)

    # gather source: the table viewed as 512B pair rows [NPAIRS, 256]
    pair_ap = AP(table[:].tensor, 0, [[2 * OUT_C, NPAIRS], [1, 2 * OUT_C]])

    S = OUT_C  # slot stride (elements) in the selected-value tile v

    def slot_ap(t, slot0, dims, stride=None):
        """AP over value tile t: partition dim + (slot_step, count) dims + c.

        stride overrides the slot stride in elements (256 for the raw pair
        buffer whose a-halves act as the stage-0 value array)."""
        ss = S if stride is None else stride
        base = t[:]
        free = [[st * ss, ct] for (st, ct) in dims if ct != 1]
        return AP(base.tensor, base.offset + slot0 * ss, [base.ap[0]] + free + [[1, OUT_C]])

    def sort_stages(H):
        ks = []
        k = 2
        while k <= H:
            j = k // 2
            while j >= 1:
                ks.append((k, j))
                j //= 2
            k *= 2
        return ks

    with tile.TileContext(nc) as tc:
        nc.gpsimd.load_library(mlp)
        with (
            tc.tile_pool(name="const", bufs=1) as cpool,
            tc.tile_pool(name="psum", bufs=2, space="PSUM") as psum_pool,
            tc.tile_pool(name="gbuf", bufs=4) as gpool,
            tc.tile_pool(name="work", bufs=2) as wpool,
            tc.tile_pool(name="mout", bufs=2) as mpool,
        ):
            # ---- phase 1: h rows = x @ w (x chunk stationary -> [node, c]) ----
            with tc.tile_pool(name="stage", bufs=1) as spool:
                lw0 = spool.tile([128, OUT_C], fp16)
                lw1 = spool.tile([128, OUT_C], fp16)
                nc.sync.dma_start(lw0[:], w[0:128, :])
                nc.sync.dma_start(lw1[:], w[128:256, :])
                xt0 = spool.tile([128, SHARD], fp16)
                xt1 = spool.tile([128, SHARD], fp16)
                nc.sync.dma_start(xt0[:], xT[0:128, :])
                nc.sync.dma_start(xt1[:], xT[128:256, :])
                hrows = spool.tile([128, NTILES, OUT_C], fp16)
                for j in range(NTILES):
                    ns = slice(j * 128, (j + 1) * 128)
                    ps = psum_pool.tile([128, OUT_C], fp32)
                    nc.tensor.matmul(ps[:], lhsT=xt0[:, ns], rhs=lw0[:], start=True, stop=False)
                    nc.tensor.matmul(ps[:], lhsT=xt1[:, ns], rhs=lw1[:], start=False, stop=True)
                    nc.scalar.copy(hrows[:, j, :], ps[:])
                nc.sync.dma_start(
                    hshard[:].rearrange("(j n) c -> n j c", n=128), hrows[:]
                )

            # ---- phase 2: AllGather shards into the table; +inf sentinels ----
            nc.gpsimd.collective_compute(
                "AllGather",
                mybir.AluOpType.bypass,
                replica_groups=[list(range(NCORES))],
                ins=[hshard[:]],
                outs=[table[0:NPAD, :]],
            )
            inft = cpool.tile([4, OUT_C], fp16)
            nc.sync.dma_start(inft[:], infs[:])
            nc.sync.dma_start(table[NPAD:NPAD + 4, :], inft[:])

            # ---- load index/mask streams; +inf constant for pad slots ----
            idx_sb = cpool.tile([128, tot_idx_cols], i16)
            par_sb = cpool.tile([128, tot_par_cols], i16)
            pick_sb = cpool.tile([128, max(1, tot_pick)], i16)
            nc.sync.dma_start(idx_sb[:], idx_d[:])
            nc.sync.dma_start(par_sb[:], par_d[:])
            nc.sync.dma_start(pick_sb[:], pick_d[:])
            inf_const = cpool.tile([128, 15 * OUT_C], fp16)
            nc.vector.memset(inf_const[:], float("inf"))

            # ---- phase 3: gather + select + sort + median per tile ----
            icol = 0      # running idx column offset
            pcol = 0      # running parity column offset
            kcol = 0      # running pick-mask column offset
            qn = 0        # dma queue rotation
            for t, (maxd, H, rs) in enumerate(sched):
                P2 = 2 * H
                buf = gpool.tile([128, MAXSLOTS, 2 * OUT_C], fp16, tag="pair")
                for s0 in range(0, maxd, GCHUNK):
                    s1 = min(s0 + GCHUNK, maxd)
                    G = (s1 - s0) * 128
                    nc.gpsimd.dma_gather(
                        buf[:, s0:s1, :],
                        pair_ap,
                        idx_sb[:, icol + s0 * 8: icol + s1 * 8],
                        G, G, 2 * OUT_C,
                        queue_num=qn, single_packet=False)
                    qn = (qn + 1) % 4
                icol += maxd * 8

                # select the wanted half of each pair IN PLACE: overwrite the
                # a-half with the b-half where the parity mask is 1; the
                # a-half positions (stride 2*OUT_C) then act as the value
                # array for the first sort stage.
                SB = 2 * OUT_C
                bb = buf[:]
                a_sel = slot_ap(buf, 0, [(1, maxd)], stride=SB)
                b_ap = AP(bb.tensor, bb.offset + OUT_C, [bb.ap[0], [SB, maxd], [1, OUT_C]])
                pp = par_sb[:]
                m_ap = AP(pp.tensor, pp.offset + pcol, [pp.ap[0], [1, maxd], [0, OUT_C]])
                pcol += maxd
                nc.vector.copy_predicated(a_sel, m_ap, b_ap)
                if maxd < P2:
                    npad = P2 - maxd
                    nc.scalar.copy(
                        slot_ap(buf, maxd, [(1, npad)], stride=SB),
                        inf_const[:, :npad * OUT_C],
                    )

                v0 = wpool.tile([128, MAXSLOTS, OUT_C], fp16, tag="v0")
                v1 = wpool.tile([128, MAXSLOTS, OUT_C], fp16, tag="v1")

                src, sstride = buf, SB
                dst = v0
                for (k, j) in sort_stages(H):
                    if k == H:
                        lo = [(2 * j, P2 // (2 * j)), (1, j)]
                        nc.vector.tensor_tensor(
                            out=slot_ap(dst, 0, lo),
                            in0=slot_ap(src, 0, lo, stride=sstride),
                            in1=slot_ap(src, j, lo, stride=sstride),
                            op=Alu.min,
                        )
                        nc.vector.tensor_tensor(
                            out=slot_ap(dst, j, lo),
                            in0=slot_ap(src, 0, lo, stride=sstride),
                            in1=slot_ap(src, j, lo, stride=sstride),
                            op=Alu.max,
                        )
                    else:
                        dims = [(2 * k, P2 // (2 * k)), (2 * j, k // (2 * j)), (1, j)]
                        for desc in (0, 1):
                            base = k if desc else 0
                            lo_out, hi_out = (j, 0) if desc else (0, j)
                            nc.vector.tensor_tensor(
                                out=slot_ap(dst, base + lo_out, dims),
                                in0=slot_ap(src, base, dims, stride=sstride),
                                in1=slot_ap(src, base + j, dims, stride=sstride),
                                op=Alu.min,
                            )
                            nc.vector.tensor_tensor(
                                out=slot_ap(dst, base + hi_out, dims),
                                in0=slot_ap(src, base, dims, stride=sstride),
                                in1=slot_ap(src, base + j, dims, stride=sstride),
                                op=Alu.max,
                            )
                    if dst is v0:
                        src, dst, sstride = v0, v1, S
                    else:
                        src, dst, sstride = v1, v0, S

                # halves sorted ascending in `src` (stride sstride):
                # L = slots 0..H-1, R = H..2H-1
                o16 = mpool.tile([128, OUT_C], fp16, tag="o16")
                for ri, r in enumerate(rs):
                    m = mpool.tile([128, K // 2 + 1, OUT_C], fp16, tag=f"m{ri}")
                    sv = src[:]
                    if r > 0:
                        # cands[t] = max(L[t], R[r-1-t]), t = 0..r-1
                        nc.vector.tensor_tensor(
                            out=slot_ap(m, 0, [(1, r)]),
                            in0=slot_ap(src, 0, [(1, r)], stride=sstride),
                            in1=AP(sv.tensor, sv.offset + (H + r - 1) * sstride,
                                   [sv.ap[0], [-sstride, r], [1, OUT_C]]),
                            op=Alu.max,
                        )
                    # cands[r] = L[r], cands[r+1] = R[r]
                    nc.vector.tensor_copy(
                        slot_ap(m, r, [(1, 2)]),
                        AP(sv.tensor, sv.offset + r * sstride,
                           [sv.ap[0], [H * sstride, 2], [1, OUT_C]]),
                    )
                    # min-reduce cands[0..r+1] into cands[0]
                    n = r + 2
                    while n > 1:
                        a = n - n // 2
                        nc.vector.tensor_tensor(
                            out=slot_ap(m, 0, [(1, n // 2)]),
                            in0=slot_ap(m, 0, [(1, n // 2)]),
                            in1=slot_ap(m, a, [(1, n // 2)]),
                            op=Alu.min,
                        )
                        n = a
                    if ri == 0:
                        nc.vector.tensor_copy(o16[:], slot_ap(m, 0, [(1, 1)]))
                    else:
                        pk = pick_sb[:]
                        pk_ap = AP(pk.tensor, pk.offset + kcol, [pk.ap[0], [0, OUT_C]])
                        kcol += 1
                        nc.vector.copy_predicated(o16[:], pk_ap, slot_ap(m, 0, [(1, 1)]))

                o32 = mpool.tile([128, OUT_C], fp32, tag="o32")
                nc.scalar.copy(o32[:], o16[:])
                nc.sync.dma_start(out[t * 128:(t + 1) * 128, :], o32[:])

    nc.compile()
    return nc


def _prepare(x, kernel, neighbors, degrees):
    """Host-side marshaling: permutation, schedule, idx/mask streams."""
    deg = np.clip(np.asarray(degrees).astype(np.int64), 1, K)
    deg_pad = np.ones(NPAD, np.int64)
    deg_pad[:N] = deg
    # dummies (N..NPAD) have deg 1 but gather only sentinels
    order = np.argsort(-deg_pad, kind="stable")        # global rank -> node id
    deg_sorted = deg_pad[order]
    sched = _make_schedule(deg_sorted)

    # table row of node u: rank j -> core j%8, local slot j//8
    ranks = np.empty(NPAD, np.int64)
    ranks[order] = np.arange(NPAD)
    rho = (ranks % NCORES) * SHARD + ranks // NCORES   # node id -> table row

    nbr = np.asarray(neighbors).astype(np.int64)
    nbr_rows = rho[nbr]                                # [N, K]
    pair_full = np.zeros((NPAD, K), np.int64)
    par_full = np.zeros((NPAD, K), np.int64)
    pair_full[:N] = nbr_rows >> 1
    par_full[:N] = nbr_rows & 1

    xf = np.zeros((NPAD, IN_C), np.float16)
    xf[:N] = np.asarray(x, np.float32).astype(np.float16)
    wf = np.asarray(kernel, np.float32).astype(np.float16)
    infs = np.full((4, OUT_C), np.inf, np.float16)

    karr = np.arange(K, dtype=np.int64)[None, :]

    in_maps = []
    node_of = np.empty((NCORES, SHARD), np.int64)
    for c in range(NCORES):
        nodes_c = order[c::NCORES]                     # local slot i -> node id
        node_of[c] = nodes_c
        d_c = deg_pad[nodes_c]                         # descending
        pair_c = pair_full[nodes_c]                    # [SHARD, K]
        par_c = par_full[nodes_c]
        valid_c = karr < d_c[:, None]                  # [SHARD, K]

        idx_parts = []
        par_parts = []
        pick_parts = []
        for t, (maxd, H, rs) in enumerate(sched):
            sl = slice(t * 128, (t + 1) * 128)
            pt = pair_c[sl, :maxd]                     # [128, maxd]
            vt = valid_c[sl, :maxd]
            stream = np.where(vt, pt, SENT_PAIR).T     # [maxd, 128] slot-major
            wrapped = np.tile(
                stream.reshape(maxd * 8, 16).T, (8, 1)
            )                                          # [128, maxd*8]
            idx_parts.append(wrapped.astype(np.int16))
            par_parts.append(
                np.where(vt, par_c[sl, :maxd], 0).astype(np.int16)
            )                                          # [128, maxd]
            r_t = (d_c[sl] - 1) // 2                   # [128]
            for r in rs[1:]:
                pick_parts.append((r_t == r).astype(np.int16)[:, None])

        idx_all = np.ascontiguousarray(np.concatenate(idx_parts, axis=1))
        par_all = np.ascontiguousarray(np.concatenate(par_parts, axis=1))
        if pick_parts:
            pick_all = np.ascontiguousarray(np.concatenate(pick_parts, axis=1))
        else:
            pick_all = np.zeros((128, 1), np.int16)
        in_maps.append({
            "xT": np.ascontiguousarray(xf[nodes_c].T),
            "w": wf,
            "idx": idx_all,
            "par": par_all,
            "pick": pick_all,
            "infs": infs,
        })

    return sched, in_maps, node_of


def kernel(x, kernel, neighbors, degrees):
    from concourse.bass_utils import run_bass_kernel_spmd

    sched, in_maps, node_of = _prepare(x, kernel, neighbors, degrees)
    if sched not in _CACHE:
        _CACHE[sched] = _emit_program(sched)
    nc = _CACHE[sched]

    res = run_bass_kernel_spmd(nc, in_maps, list(range(NCORES)))
    full = np.empty((NPAD, OUT_C), np.float32)
    for c in range(NCORES):
        full[node_of[c]] = res.results[c]["out"]
    return np.ascontiguousarray(full[:N])
